# revision 16
# baseline (speedup 1.0000x reference)
"""Trainium2 Bass kernel for nn_CausalSelfAttention (B=1, S=2048, D=1024, H=16).

Tensor-parallel over heads across 8 NeuronCores: core c computes heads
(2c, 2c+1) end-to-end.  The host sums the 8 partial outputs (row-parallel
Wout) and returns (y, v1) like the reference.

v2 design (vs the f32r v1 baseline at ~224us):
  - bf16 storage + bf16 matmuls everywhere (PSUM accumulation stays f32):
    halves DMA traffic, doubles DVE throughput, removes the f32r
    narrow-matmul penalty.  Numerics have ~100x headroom vs the 2e-2 gate.
  - phase A (QKV+norm+rope) is S-chunk pipelined (4 chunks of 512) with
    the norm/rope chain of chunk c emitted during chunk c+1's projection
    matmuls, so the PE never waits on the DVE/ACT latency chain.
  - both rms-norm scales are pre-folded into q/k (q also gets 1/sqrt(hd)),
    so the softmax exp needs no scale AP and runs as ONE merged-head ACT
    instruction per (strip, key-block): [128, 2, <=512].
  - phase B is query-strip-outer (4 strips of 512) flash-style: per strip,
    scores -> exp -> PV accumulate over key blocks; causal mask added by a
    DVE add on the diagonal block; softmax denominator from a ones column
    in the PV stationary; out-projection + output DMA of strip i-1
    interleaved into strip i to fill PE bubbles and stream the output.
  - first matmul starts ~2.5us in (v1 waited 22us for the full f32 xT).
"""

import os
import sys

import numpy as np

try:
    import concourse.bass as bass  # noqa: F401
except Exception:  # pragma: no cover
    for _p in ("/opt/trn_rl_repo", "/root/.axon_site/_ro/trn_rl_repo"):
        if os.path.isdir(_p) and _p not in sys.path:
            sys.path.insert(0, _p)

import concourse.bacc as bacc
import concourse.bass as bass
import concourse.mybir as mybir
import concourse.tile as tile
from concourse import bass_utils

S = 2048
D = 1024
SCHRAUD = True           # route some non-diag exp blocks to DVE (bit-trick exp)
SA = 184.6649652         # 128 * log2(e)
SB = 16249.17            # 127*128 - 7.33 + 0.5 (calibrated for truncating cast)
NH = 16
HD = 64
NCORES = 8
NKC = D // 128           # 8 contraction chunks for the projections
CH = 512                 # S-chunk width (phase A) == query-strip width (phase B)
NCH = S // CH            # 4
NB = S // 128            # 16 key blocks

F32 = mybir.dt.float32
F32R = mybir.dt.float32r
BF16 = mybir.dt.bfloat16
AF = mybir.ActivationFunctionType

EPS = float(np.finfo(np.float32).eps)
NEG = -1e30


def r(ap):
    return ap.bitcast(F32R)


def _emit(tc, io, dbg=False):
    nc = tc.nc
    pools = []

    def pool(*a, **k):
        p = tc.alloc_tile_pool(*a, **k)
        pools.append(p)
        return p

    def release(p):
        pools.remove(p)
        p.release()

    consts = pool(name="consts", bufs=1)
    wpool = pool(name="wpool", bufs=1)
    persist = pool(name="persist", bufs=1)
    work = pool(name="work", bufs=2)
    late = pool(name="late", bufs=1)

    # ---- SBUF constants / weights -----------------------------------
    identb = consts.tile([128, 128], BF16)
    maskf2 = consts.tile([128, 2, 128], F32)
    c4q = consts.tile([2, 2], F32)
    c4k = consts.tile([2, 2], F32)
    ind8 = consts.tile([128, 4], BF16)
    indT2 = consts.tile([2, 128], BF16)
    ones64 = consts.tile([1, 64], BF16)
    cosT = consts.tile([128, S], BF16)
    sinTs = consts.tile([128, S], BF16)

    w_sb = {}
    for nm in ("wq", "wk", "wv"):
        w_sb[nm] = wpool.tile([128, NKC, 128], BF16, name=nm)
    wo_sb = wpool.tile([128, D], BF16)
    v1s = wpool.tile([128, S], BF16)
    xt = wpool.tile([128, NKC, S], BF16)

    q_fin = persist.tile([128, S], BF16)
    k_fin = persist.tile([128, S], BF16)
    vT = persist.tile([128, S], BF16)
    v_ext = persist.tile([128, NB, 130], BF16)
    y2T = persist.tile([128, S], BF16)

    # ---- DMA issue order: sync ring carries the PE-critical stream,
    # gpsimd ring carries v1/cos/sin (+ the rope swaps emitted later) ----
    xt_dram = io["xT"].ap().rearrange("(po pi) s -> pi po s", pi=128)
    nc.sync.dma_start(out=w_sb["wq"], in_=io["wqP"].ap())
    nc.sync.dma_start(out=xt[:, 0:4, 0:CH], in_=xt_dram[:, 0:4, 0:CH])
    nc.sync.dma_start(out=xt[:, 4:8, 0:CH], in_=xt_dram[:, 4:8, 0:CH])
    nc.sync.dma_start(out=w_sb["wk"], in_=io["wkP"].ap())
    nc.sync.dma_start(out=w_sb["wv"], in_=io["wvP"].ap())
    nc.sync.dma_start(out=xt[:, :, CH:2 * CH], in_=xt_dram[:, :, CH:2 * CH])
    nc.sync.dma_start(out=v1s, in_=io["v1Ts"].ap())
    nc.sync.dma_start(out=cosT, in_=io["cosT"].ap())
    nc.sync.dma_start(out=sinTs, in_=io["sinTs"].ap())
    nc.sync.dma_start(out=xt[:, :, 2 * CH:3 * CH], in_=xt_dram[:, :, 2 * CH:3 * CH])
    nc.sync.dma_start(out=xt[:, :, 3 * CH:4 * CH], in_=xt_dram[:, :, 3 * CH:4 * CH])
    nc.sync.dma_start(out=wo_sb, in_=io["woT"].ap())
    for t, nm in ((ind8, "ind8"), (indT2, "indT2"), (identb, "identb"),
                  (c4q, "c4q"), (c4k, "c4k"), (ones64, "ones64"),
                  (maskf2, "maskf2")):
        nc.gpsimd.dma_start(out=t, in_=io[nm].ap())
    nc.vector.memset(v_ext[:, :, 64:65], 1.0)
    nc.vector.memset(v_ext[:, :, 129:130], 1.0)

    # ================= phase A: QKV + norm + rope =====================
    pa_proj = pool(name="pa_proj", bufs=3, space="PSUM")
    pa_norm = pool(name="pa_norm", bufs=2, space="PSUM")
    pa_bc = pool(name="pa_bc", bufs=2, space="PSUM")
    pa_vt = pool(name="pa_vt", bufs=1, space="PSUM")

    raw = {}    # c -> (qr, kr)
    sqs = {}    # c -> (sqq, sqk)
    swps = {}   # c -> (swq, swk)

    def proj(c, which):
        s0 = CH * c
        ps = pa_proj.tile([128, CH], F32, tag="proj", name=f"ps_{which}{c}")
        w = w_sb["w" + which]
        for kc in range(NKC):
            nc.tensor.matmul(ps, w[:, kc, :], xt[:, kc, s0:s0 + CH],
                             start=(kc == 0), stop=(kc == NKC - 1))
        if which == "v":
            nc.vector.tensor_add(out=vT[:, s0:s0 + CH], in0=ps,
                                 in1=v1s[:, s0:s0 + CH])
            return
        tr = work.tile([128, CH], BF16, tag="raw" + which, name=f"{which}r{c}")
        nc.scalar.copy(out=tr, in_=ps)                      # ACT evac
        sq = work.tile([128, CH], BF16, tag="sq" + which, name=f"sq{which}{c}")
        nc.vector.tensor_mul(out=sq, in0=tr, in1=tr)        # DVE square (2x bf16)
        sw = work.tile([128, CH], BF16, tag="sw" + which, name=f"sw{which}{c}")
        # rope-partner swap (0..31 <-> 32..63 within each 64-dim head)
        for d0, sp in ((0, 32), (32, 0), (64, 96), (96, 64)):
            nc.gpsimd.dma_start(out=sw[d0:d0 + 32, :], in_=tr[sp:sp + 32, :])
        if which == "q":
            raw[c] = [tr, None]
            sqs[c] = [sq, None]
            swps[c] = [sw, None]
        else:
            raw[c][1] = tr
            sqs[c][1] = sq
            swps[c][1] = sw

    def normchain(c):
        s0 = CH * c
        sqq, sqk = sqs[c]
        ps_nq = pa_norm.tile([2, CH], F32, tag="n", name=f"nq{c}")
        nc.tensor.matmul(ps_nq, ind8[:, 0:2], sqq, start=True, stop=True)
        ps_nk = pa_norm.tile([2, CH], F32, tag="n", name=f"nk{c}")
        nc.tensor.matmul(ps_nk, ind8[:, 2:4], sqk, start=True, stop=True)
        sq_q4 = work.tile([2, CH], F32, tag="sq4q", name=f"sq4q_{c}")
        nc.scalar.activation(out=sq_q4, in_=ps_nq, func=AF.Sqrt,
                             bias=c4q[:, 1:2], scale=c4q[:, 0:1])
        sq_k4 = work.tile([2, CH], F32, tag="sq4k", name=f"sq4k_{c}")
        nc.scalar.activation(out=sq_k4, in_=ps_nk, func=AF.Sqrt,
                             bias=c4k[:, 1:2], scale=c4k[:, 0:1])
        invq = work.tile([2, CH], F32, tag="invq", name=f"invq_{c}")
        nc.vector.reciprocal_approx_fast(out=invq, in_=sq_q4)
        invk = work.tile([2, CH], F32, tag="invk", name=f"invk_{c}")
        nc.vector.reciprocal_approx_fast(out=invk, in_=sq_k4)
        invqb = work.tile([2, CH], BF16, tag="invqb", name=f"invqb_{c}")
        nc.scalar.copy(out=invqb, in_=invq)
        invkb = work.tile([2, CH], BF16, tag="invkb", name=f"invkb_{c}")
        nc.scalar.copy(out=invkb, in_=invk)
        rq = pa_bc.tile([128, CH], F32, tag="bc", name=f"rq{c}")
        nc.tensor.matmul(rq, indT2, invqb, start=True, stop=True)
        rk = pa_bc.tile([128, CH], F32, tag="bc", name=f"rk{c}")
        nc.tensor.matmul(rk, indT2, invkb, start=True, stop=True)
        for x, (tr, sw, rr, fin) in enumerate(
                ((raw[c][0], swps[c][0], rq, q_fin),
                 (raw[c][1], swps[c][1], rk, k_fin))):
            nc.vector.tensor_mul(out=sw, in0=sw, in1=sinTs[:, s0:s0 + CH])
            nc.vector.tensor_mul(out=tr, in0=tr, in1=cosT[:, s0:s0 + CH])
            nc.vector.tensor_add(out=tr, in0=tr, in1=sw)
            nc.vector.tensor_mul(out=fin[:, s0:s0 + CH], in0=tr, in1=rr)
        for t in range(4):
            tb = 4 * c + t
            ps_vt = pa_vt.tile([128, 128], BF16, tag="vt", name=f"vt{tb}")
            nc.tensor.transpose(ps_vt, vT[:, 128 * tb:128 * tb + 128], identb)
            dst = v_ext[:, tb, 0:130].rearrange("p (a c) -> p a c", a=2)[:, :, 0:64]
            src = ps_vt.rearrange("p (a c) -> p a c", c=64)
            if t % 2 == 0:
                nc.vector.tensor_copy(out=dst, in_=src)
            else:
                nc.scalar.copy(out=dst, in_=src)

    for c in range(NCH):
        proj(c, "q")
        if c > 0:
            normchain(c - 1)
        proj(c, "k")
        proj(c, "v")
    normchain(NCH - 1)

    # ================= phase B: attention + out-proj ==================
    release(pa_vt)
    release(pa_bc)
    release(pa_norm)
    release(pa_proj)

    pb_sc = pool(name="pb_sc", bufs=3, space="PSUM")
    pb_yt = pool(name="pb_yt", bufs=2, space="PSUM")
    outp = io["outp"].ap()

    def make_oproj(i):
        def em(oc, i=i):
            po = pb_sc.tile([128, CH], F32, tag="sc", name=f"po{i}_{oc}")
            nc.tensor.matmul(po, wo_sb[:, 128 * oc:128 * oc + 128],
                             y2T[:, CH * i:CH * i + CH], start=True, stop=True)
            ob = late.tile([128, CH], BF16, tag="osb", bufs=3,
                           name=f"osb{i}_{oc}")
            if oc % 2 == 0:
                nc.vector.tensor_copy(out=ob, in_=po)
                nc.sync.dma_start(out=outp[oc][:, CH * i:CH * i + CH], in_=ob)
            else:
                nc.scalar.copy(out=ob, in_=po)
                nc.gpsimd.dma_start(out=outp[oc][:, CH * i:CH * i + CH], in_=ob)
        return [lambda oc=oc: em(oc) for oc in range(8)]

    pending = []
    for i in range(NCH):
        yts = [pb_yt.tile([65, CH], F32, tag="yt", name=f"yt{i}_{h}")
               for h in (0, 1)]
        nj = 4 * i + 4

        # diagonal blocks first: their DVE mask-adds run while DVE is idle,
        # and the strip tail becomes a pure PE->ACT stream.
        order = list(range(4 * i, nj)) + list(range(0, 4 * i))

        def emit_pv(idx, j, ex, s0, yts=yts, i=i, nj=nj):
            first = (idx == 0)
            last_nd = (i > 0 and j == 4 * i - 1)
            for h in (0, 1):
                vst = v_ext[:, j, 65 * h:65 * h + 65]
                if i == 0:
                    # all-diag strip: stop on each leading 128 window
                    nc.tensor.matmul(yts[h][:, s0:s0 + 128], vst,
                                     ex[:, h, s0:s0 + 128],
                                     start=first, stop=True)
                    if s0 + 128 < CH:
                        nc.tensor.matmul(yts[h][:, s0 + 128:CH], vst,
                                         ex[:, h, s0 + 128:CH],
                                         start=first, stop=False)
                else:
                    nc.tensor.matmul(yts[h][:, s0:CH], vst, ex[:, h, s0:CH],
                                     start=first, stop=last_nd)

        prev = None
        for idx, j in enumerate(order):
            diag = (j - 4 * i) if j >= 4 * i else -1
            s0 = 128 * diag if diag >= 0 else 0
            sc = pb_sc.tile([128, 2, CH], F32, tag="sc", name=f"sc{i}_{j}")
            for h in (0, 1):
                nc.tensor.matmul(sc[:, h, s0:CH],
                                 k_fin[64 * h:64 * h + 64, 128 * j:128 * j + 128],
                                 q_fin[64 * h:64 * h + 64, CH * i + s0:CH * i + CH],
                                 start=True, stop=True)
            if diag >= 0:
                nc.vector.tensor_add(out=sc[:, :, s0:s0 + 128],
                                     in0=sc[:, :, s0:s0 + 128], in1=maskf2)
            ex = late.tile([128, 2, CH], BF16, tag="ex", bufs=3,
                           name=f"ex{i}_{j}")
            if SCHRAUD and diag < 0 and j % 3 == 2:
                nc.vector.tensor_scalar(
                    out=ex.bitcast(mybir.dt.int16), in0=sc,
                    scalar1=SA, scalar2=SB,
                    op0=mybir.AluOpType.mult, op1=mybir.AluOpType.add)
            else:
                nc.scalar.activation(out=ex[:, :, s0:CH], in_=sc[:, :, s0:CH],
                                     func=AF.Exp)
            if prev is not None:
                emit_pv(*prev)
            prev = (idx, j, ex, s0)
            if idx >= 2:
                for _ in range(2):
                    if pending:
                        pending.pop(0)()
        emit_pv(*prev)
        while pending:
            pending.pop(0)()

        invdbs = []
        for h in (0, 1):
            dens = late.tile([1, CH], F32, tag="dens", bufs=2,
                             name=f"dens{i}_{h}")
            nc.vector.tensor_copy(out=dens, in_=yts[h][64:65, :])
            invd = late.tile([1, CH], F32, tag="invd", bufs=2,
                             name=f"ivd{i}_{h}")
            nc.vector.reciprocal_approx_fast(out=invd, in_=dens)
            if dbg:
                nc.sync.dma_start(out=io["dbg_invd"].ap()[2 * i + h], in_=invd)
            invdb = late.tile([1, CH], BF16, tag="invdb", bufs=2,
                              name=f"ivdb{i}_{h}")
            nc.vector.tensor_copy(out=invdb, in_=invd)
            invdbs.append(invdb)

        def denfin(h, i=i, yts=yts, invdbs=invdbs):
            ib = pb_sc.tile([64, CH], F32, tag="sc", name=f"ib{i}_{h}")
            nc.tensor.matmul(ib, ones64, invdbs[h], start=True, stop=True)
            ibs = late.tile([64, CH], F32, tag="ibs", bufs=2, name=f"ibs{i}_{h}")
            nc.scalar.copy(out=ibs, in_=ib)
            nc.vector.tensor_mul(out=y2T[64 * h:64 * h + 64, CH * i:CH * i + CH],
                                 in0=yts[h][0:64, :], in1=ibs)

        pending = [lambda h=h: denfin(h) for h in (0, 1)] + make_oproj(i)

    for em in pending:
        em()

    if dbg:
        nc.sync.dma_start(out=io["dbg_q"].ap(), in_=q_fin)
        nc.sync.dma_start(out=io["dbg_k"].ap(), in_=k_fin)
        nc.sync.dma_start(out=io["dbg_vext"].ap(), in_=v_ext)
        nc.sync.dma_start(out=io["dbg_y2T"].ap(), in_=y2T)

    release(pb_yt)
    release(pb_sc)
    for p in reversed(pools):
        p.release()


_CACHE = {}


def _build(debug_taps=False):
    key = ("nc", debug_taps)
    if key in _CACHE:
        return _CACHE[key]
    nc = bacc.Bacc("TRN2", target_bir_lowering=False, debug=False,
                   enable_asserts=True, num_devices=NCORES)
    io = {}
    io["xT"] = nc.dram_tensor("xT", [D, S], BF16, kind="ExternalInput")
    io["cosT"] = nc.dram_tensor("cosT", [128, S], BF16, kind="ExternalInput")
    io["sinTs"] = nc.dram_tensor("sinTs", [128, S], BF16, kind="ExternalInput")
    io["identb"] = nc.dram_tensor("identb", [128, 128], BF16, kind="ExternalInput")
    io["maskf2"] = nc.dram_tensor("maskf2", [128, 2, 128], F32, kind="ExternalInput")
    io["c4q"] = nc.dram_tensor("c4q", [2, 2], F32, kind="ExternalInput")
    io["c4k"] = nc.dram_tensor("c4k", [2, 2], F32, kind="ExternalInput")
    io["ind8"] = nc.dram_tensor("ind8", [128, 4], BF16, kind="ExternalInput")
    io["indT2"] = nc.dram_tensor("indT2", [2, 128], BF16, kind="ExternalInput")
    io["ones64"] = nc.dram_tensor("ones64", [1, 64], BF16, kind="ExternalInput")
    io["wqP"] = nc.dram_tensor("wqP", [128, NKC, 128], BF16, kind="ExternalInput")
    io["wkP"] = nc.dram_tensor("wkP", [128, NKC, 128], BF16, kind="ExternalInput")
    io["wvP"] = nc.dram_tensor("wvP", [128, NKC, 128], BF16, kind="ExternalInput")
    io["woT"] = nc.dram_tensor("woT", [128, D], BF16, kind="ExternalInput")
    io["v1Ts"] = nc.dram_tensor("v1Ts", [128, S], BF16, kind="ExternalInput")
    io["outp"] = nc.dram_tensor("outp", [8, 128, S], BF16, kind="ExternalOutput")
    if debug_taps:
        io["dbg_q"] = nc.dram_tensor("dbg_q", [128, S], BF16, kind="ExternalOutput")
        io["dbg_k"] = nc.dram_tensor("dbg_k", [128, S], BF16, kind="ExternalOutput")
        io["dbg_vext"] = nc.dram_tensor("dbg_vext", [128, NB, 130], BF16, kind="ExternalOutput")
        io["dbg_y2T"] = nc.dram_tensor("dbg_y2T", [128, S], BF16, kind="ExternalOutput")
        io["dbg_invd"] = nc.dram_tensor("dbg_invd", [8, 1, CH], F32, kind="ExternalOutput")
        for j in (0, 4):
            io[f"dbg_ex{j}"] = nc.dram_tensor(f"dbg_ex{j}", [128, 2, CH], BF16, kind="ExternalOutput")
            io[f"dbg_sc{j}"] = nc.dram_tensor(f"dbg_sc{j}", [128, 2, CH], F32, kind="ExternalOutput")

    with tile.TileContext(nc) as tc:
        _emit(tc, io, dbg=debug_taps)
    nc.compile()
    _CACHE[key] = nc
    return nc


def _host_prep(x, v1, Wq, Wk, Wv, Wout, lambdas):
    """Build per-core input maps (bf16 numpy)."""
    import ml_dtypes
    BF = ml_dtypes.bfloat16

    x = np.asarray(x, np.float32).reshape(S, D)
    v1 = np.asarray(v1, np.float32).reshape(S, D)
    Wq = np.asarray(Wq, np.float32)
    Wk = np.asarray(Wk, np.float32)
    Wv = np.asarray(Wv, np.float32)
    Wout = np.asarray(Wout, np.float32)
    lam = np.float32(np.asarray(lambdas))

    xT = np.ascontiguousarray(x.T).astype(BF)

    inv_freq = (np.float32(1.0)
                / np.power(np.float32(10000.0),
                           np.arange(0, HD, 2, dtype=np.float32) / np.float32(HD)))
    t = np.arange(S, dtype=np.float32)
    freqs = np.outer(t, inv_freq).astype(np.float32)        # [S, 32]
    cos = np.cos(freqs).T                                    # [32, S]
    sin = np.sin(freqs).T
    cosT = np.ascontiguousarray(np.tile(cos, (4, 1))).astype(BF)
    sinTs = np.ascontiguousarray(
        np.concatenate([sin, -sin, sin, -sin], axis=0)).astype(BF)

    identb = np.eye(128, dtype=BF)
    kk, qq = np.meshgrid(np.arange(128), np.arange(128), indexing="ij")
    mask1 = np.where(qq >= kk, 0.0, NEG).astype(np.float32)
    maskf2 = np.ascontiguousarray(
        np.broadcast_to(mask1[:, None, :], (128, 2, 128))).astype(np.float32)

    # sqrt(scale*sum + bias): rows 0-1 q -> 8*rms_q, rows 2-3 k -> rms_k
    c4q = np.array([[1.0, 64.0 * EPS],
                    [1.0, 64.0 * EPS]], dtype=np.float32)
    c4k = np.array([[1.0 / 64.0, EPS],
                    [1.0 / 64.0, EPS]], dtype=np.float32)
    ind8 = np.zeros((128, 4), dtype=BF)
    ind8[0:64, 0] = 1.0
    ind8[64:128, 1] = 1.0
    ind8[0:64, 2] = 1.0
    ind8[64:128, 3] = 1.0
    indT2 = np.zeros((2, 128), dtype=BF)
    indT2[0, 0:64] = 1.0
    indT2[1, 64:128] = 1.0
    ones64 = np.ones((1, 64), dtype=BF)

    shared = dict(xT=xT, cosT=cosT, sinTs=sinTs, identb=identb, maskf2=maskf2,
                  c4q=c4q, c4k=c4k, ind8=ind8, indT2=indT2, ones64=ones64)

    in_maps = []
    for c in range(NCORES):
        sl = slice(128 * c, 128 * c + 128)
        m = dict(shared)
        def prearr(wt):
            # [D, 128] -> [pi=128, po=8, m=128] so DMA runs are 2KB/partition
            return np.ascontiguousarray(
                wt.reshape(NKC, 128, 128).transpose(1, 0, 2)).astype(BF)

        m["wqP"] = prearr(Wq[sl, :].T)
        m["wkP"] = prearr(Wk[sl, :].T)
        m["wvP"] = prearr(((np.float32(1.0) - lam) * Wv[sl, :]).T)
        m["woT"] = np.ascontiguousarray(Wout[:, sl].T).astype(BF)
        m["v1Ts"] = np.ascontiguousarray((lam * v1[:, sl]).T).astype(BF)
        in_maps.append(m)
    return in_maps


def run(inputs, trace=False, debug_taps=False):
    nh = int(np.asarray(inputs["n_heads"]))
    assert nh == NH, f"kernel compiled for n_heads={NH}, got {nh}"
    nc = _build(debug_taps)
    in_maps = _host_prep(inputs["x"], inputs["v1"], inputs["Wq"], inputs["Wk"],
                         inputs["Wv"], inputs["Wout"], inputs["lambdas"])
    res = bass_utils.run_bass_kernel_spmd(
        nc, in_maps, core_ids=list(range(NCORES)), trace=trace)
    outT = np.zeros((D, S), dtype=np.float32)
    for c in range(NCORES):
        outT += np.asarray(res.results[c]["outp"], dtype=np.float32).reshape(D, S)
    y = np.ascontiguousarray(outT.T).reshape(1, S, D).astype(np.float32)
    v1 = np.asarray(inputs["v1"], np.float32).reshape(1, S, D)
    return (y, v1), res


def kernel(**inputs):
    outs, _ = run(inputs, trace=False)
    return outs


# revision 17
# speedup vs baseline: 1.1214x; 1.1214x over previous
"""Trainium2 Bass kernel for nn_CausalSelfAttention (B=1, S=2048, D=1024, H=16).

Tensor-parallel over heads across 8 NeuronCores: core c computes heads
(2c, 2c+1) end-to-end.  The host sums the 8 partial outputs (row-parallel
Wout) and returns (y, v1) like the reference.

v2 design (vs the f32r v1 baseline at ~224us):
  - bf16 storage + bf16 matmuls everywhere (PSUM accumulation stays f32):
    halves DMA traffic, doubles DVE throughput, removes the f32r
    narrow-matmul penalty.  Numerics have ~100x headroom vs the 2e-2 gate.
  - phase A (QKV+norm+rope) is S-chunk pipelined (4 chunks of 512) with
    the norm/rope chain of chunk c emitted during chunk c+1's projection
    matmuls, so the PE never waits on the DVE/ACT latency chain.
  - both rms-norm scales are pre-folded into q/k (q also gets 1/sqrt(hd)),
    so the softmax exp needs no scale AP and runs as ONE merged-head ACT
    instruction per (strip, key-block): [128, 2, <=512].
  - phase B is query-strip-outer (4 strips of 512) flash-style: per strip,
    scores -> exp -> PV accumulate over key blocks; causal mask added by a
    DVE add on the diagonal block; softmax denominator from a ones column
    in the PV stationary; out-projection + output DMA of strip i-1
    interleaved into strip i to fill PE bubbles and stream the output.
  - first matmul starts ~2.5us in (v1 waited 22us for the full f32 xT).
"""

import os
import sys

import numpy as np

try:
    import concourse.bass as bass  # noqa: F401
except Exception:  # pragma: no cover
    for _p in ("/opt/trn_rl_repo", "/root/.axon_site/_ro/trn_rl_repo"):
        if os.path.isdir(_p) and _p not in sys.path:
            sys.path.insert(0, _p)

import concourse.bacc as bacc
import concourse.bass as bass
import concourse.mybir as mybir
import concourse.tile as tile
from concourse import bass_utils

S = 2048
D = 1024
SCHRAUD = True           # route some non-diag exp blocks to DVE (bit-trick exp)
SA = 184.6649652         # 128 * log2(e)
SB = 16249.17            # 127*128 - 7.33 + 0.5 (calibrated for truncating cast)
NH = 16
HD = 64
NCORES = 8
NKC = D // 128           # 8 contraction chunks for the projections
CH = 512                 # S-chunk width (phase A) == query-strip width (phase B)
NCH = S // CH            # 4
NB = S // 128            # 16 key blocks

F32 = mybir.dt.float32
F32R = mybir.dt.float32r
BF16 = mybir.dt.bfloat16
AF = mybir.ActivationFunctionType

EPS = float(np.finfo(np.float32).eps)
NEG = -80.0


def r(ap):
    return ap.bitcast(F32R)


def _emit(tc, io, dbg=False):
    nc = tc.nc
    pools = []

    def pool(*a, **k):
        p = tc.alloc_tile_pool(*a, **k)
        pools.append(p)
        return p

    def release(p):
        pools.remove(p)
        p.release()

    consts = pool(name="consts", bufs=1)
    wpool = pool(name="wpool", bufs=1)
    persist = pool(name="persist", bufs=1)
    work = pool(name="work", bufs=2)
    late = pool(name="late", bufs=1)

    # ---- SBUF constants / weights -----------------------------------
    identb = consts.tile([128, 128], BF16)
    maskb = consts.tile([128, 128], BF16)
    c4q = consts.tile([2, 2], F32)
    c4k = consts.tile([2, 2], F32)
    ind8 = consts.tile([128, 4], BF16)
    indT2 = consts.tile([2, 128], BF16)
    ones64 = consts.tile([1, 64], BF16)
    cosT = consts.tile([128, S], BF16)
    sinTs = consts.tile([128, S], BF16)

    w_sb = {}
    for nm in ("wq", "wk", "wv"):
        w_sb[nm] = wpool.tile([128, NKC, 128], BF16, name=nm)
    wo_sb = wpool.tile([128, D], BF16)
    v1s = wpool.tile([128, S], BF16)
    xt = wpool.tile([128, NKC, S], BF16)

    q_fin = persist.tile([128, S], BF16)
    k_fin = persist.tile([128, S], BF16)
    vT = persist.tile([128, S], BF16)
    v_ext = persist.tile([128, NB, 130], BF16)
    y2T = persist.tile([128, S], BF16)

    # ---- DMA issue order: sync ring carries the PE-critical stream,
    # gpsimd ring carries v1/cos/sin (+ the rope swaps emitted later) ----
    xt_dram = io["xT"].ap().rearrange("(po pi) s -> pi po s", pi=128)
    nc.sync.dma_start(out=w_sb["wq"], in_=io["wqP"].ap())
    nc.sync.dma_start(out=xt[:, 0:4, 0:CH], in_=xt_dram[:, 0:4, 0:CH])
    nc.sync.dma_start(out=xt[:, 4:8, 0:CH], in_=xt_dram[:, 4:8, 0:CH])
    nc.sync.dma_start(out=w_sb["wk"], in_=io["wkP"].ap())
    nc.sync.dma_start(out=w_sb["wv"], in_=io["wvP"].ap())
    nc.sync.dma_start(out=xt[:, :, CH:2 * CH], in_=xt_dram[:, :, CH:2 * CH])
    nc.sync.dma_start(out=v1s, in_=io["v1Ts"].ap())
    nc.sync.dma_start(out=cosT, in_=io["cosT"].ap())
    nc.sync.dma_start(out=sinTs, in_=io["sinTs"].ap())
    nc.sync.dma_start(out=xt[:, :, 2 * CH:3 * CH], in_=xt_dram[:, :, 2 * CH:3 * CH])
    nc.sync.dma_start(out=xt[:, :, 3 * CH:4 * CH], in_=xt_dram[:, :, 3 * CH:4 * CH])
    nc.sync.dma_start(out=wo_sb, in_=io["woT"].ap())
    for t, nm in ((ind8, "ind8"), (indT2, "indT2"), (identb, "identb"),
                  (c4q, "c4q"), (c4k, "c4k"), (ones64, "ones64"),
                  (maskb, "maskb")):
        nc.gpsimd.dma_start(out=t, in_=io[nm].ap())
    nc.vector.memset(v_ext[:, :, 64:65], 1.0)
    nc.vector.memset(v_ext[:, :, 129:130], 1.0)

    # ================= phase A: QKV + norm + rope =====================
    pa_proj = pool(name="pa_proj", bufs=3, space="PSUM")
    pa_norm = pool(name="pa_norm", bufs=2, space="PSUM")
    pa_bc = pool(name="pa_bc", bufs=2, space="PSUM")
    pa_vt = pool(name="pa_vt", bufs=1, space="PSUM")

    raw = {}    # c -> (qr, kr)
    sqs = {}    # c -> (sqq, sqk)
    swps = {}   # c -> (swq, swk)

    def proj(c, which):
        s0 = CH * c
        ps = pa_proj.tile([128, CH], F32, tag="proj", name=f"ps_{which}{c}")
        w = w_sb["w" + which]
        for kc in range(NKC):
            nc.tensor.matmul(ps, w[:, kc, :], xt[:, kc, s0:s0 + CH],
                             start=(kc == 0), stop=(kc == NKC - 1))
        if which == "v":
            nc.vector.tensor_add(out=vT[:, s0:s0 + CH], in0=ps,
                                 in1=v1s[:, s0:s0 + CH])
            return
        tr = work.tile([128, CH], BF16, tag="raw" + which, name=f"{which}r{c}")
        nc.scalar.copy(out=tr, in_=ps)                      # ACT evac
        sq = work.tile([128, CH], BF16, tag="sq" + which, name=f"sq{which}{c}")
        nc.vector.tensor_mul(out=sq, in0=tr, in1=tr)        # DVE square (2x bf16)
        sw = work.tile([128, CH], BF16, tag="sw" + which, name=f"sw{which}{c}")
        # rope-partner swap (0..31 <-> 32..63 within each 64-dim head)
        for d0, sp in ((0, 32), (32, 0), (64, 96), (96, 64)):
            nc.gpsimd.dma_start(out=sw[d0:d0 + 32, :], in_=tr[sp:sp + 32, :])
        if which == "q":
            raw[c] = [tr, None]
            sqs[c] = [sq, None]
            swps[c] = [sw, None]
        else:
            raw[c][1] = tr
            sqs[c][1] = sq
            swps[c][1] = sw

    def normchain(c):
        s0 = CH * c
        sqq, sqk = sqs[c]
        ps_nq = pa_norm.tile([2, CH], F32, tag="n", name=f"nq{c}")
        nc.tensor.matmul(ps_nq, ind8[:, 0:2], sqq, start=True, stop=True)
        ps_nk = pa_norm.tile([2, CH], F32, tag="n", name=f"nk{c}")
        nc.tensor.matmul(ps_nk, ind8[:, 2:4], sqk, start=True, stop=True)
        sq_q4 = work.tile([2, CH], F32, tag="sq4q", name=f"sq4q_{c}")
        nc.scalar.activation(out=sq_q4, in_=ps_nq, func=AF.Sqrt,
                             bias=c4q[:, 1:2], scale=c4q[:, 0:1])
        sq_k4 = work.tile([2, CH], F32, tag="sq4k", name=f"sq4k_{c}")
        nc.scalar.activation(out=sq_k4, in_=ps_nk, func=AF.Sqrt,
                             bias=c4k[:, 1:2], scale=c4k[:, 0:1])
        invq = work.tile([2, CH], F32, tag="invq", name=f"invq_{c}")
        nc.vector.reciprocal_approx_fast(out=invq, in_=sq_q4)
        invk = work.tile([2, CH], F32, tag="invk", name=f"invk_{c}")
        nc.vector.reciprocal_approx_fast(out=invk, in_=sq_k4)
        invqb = work.tile([2, CH], BF16, tag="invqb", name=f"invqb_{c}")
        nc.scalar.copy(out=invqb, in_=invq)
        invkb = work.tile([2, CH], BF16, tag="invkb", name=f"invkb_{c}")
        nc.scalar.copy(out=invkb, in_=invk)
        rq = pa_bc.tile([128, CH], F32, tag="bc", name=f"rq{c}")
        nc.tensor.matmul(rq, indT2, invqb, start=True, stop=True)
        rk = pa_bc.tile([128, CH], F32, tag="bc", name=f"rk{c}")
        nc.tensor.matmul(rk, indT2, invkb, start=True, stop=True)
        for x, (tr, sw, rr, fin) in enumerate(
                ((raw[c][0], swps[c][0], rq, q_fin),
                 (raw[c][1], swps[c][1], rk, k_fin))):
            nc.vector.tensor_mul(out=sw, in0=sw, in1=sinTs[:, s0:s0 + CH])
            nc.vector.tensor_mul(out=tr, in0=tr, in1=cosT[:, s0:s0 + CH])
            nc.vector.tensor_add(out=tr, in0=tr, in1=sw)
            nc.vector.tensor_mul(out=fin[:, s0:s0 + CH], in0=tr, in1=rr)
        for t in range(4):
            tb = 4 * c + t
            ps_vt = pa_vt.tile([128, 128], BF16, tag="vt", name=f"vt{tb}")
            nc.tensor.transpose(ps_vt, vT[:, 128 * tb:128 * tb + 128], identb)
            dst = v_ext[:, tb, 0:130].rearrange("p (a c) -> p a c", a=2)[:, :, 0:64]
            src = ps_vt.rearrange("p (a c) -> p a c", c=64)
            if t % 2 == 0:
                nc.vector.tensor_copy(out=dst, in_=src)
            else:
                nc.scalar.copy(out=dst, in_=src)

    for c in range(NCH):
        proj(c, "q")
        if c > 0:
            normchain(c - 1)
        proj(c, "k")
        proj(c, "v")
    normchain(NCH - 1)

    # ================= phase B: attention + out-proj ==================
    release(pa_vt)
    release(pa_bc)
    release(pa_norm)
    release(pa_proj)

    pb_sc = pool(name="pb_sc", bufs=3, space="PSUM")
    pb_yt = pool(name="pb_yt", bufs=2, space="PSUM")
    outp = io["outp"].ap()

    outp_r = outp.rearrange("o p s -> p o s")

    def make_oproj(i):
        ob = late.tile([128, 8, CH], BF16, tag="osb", bufs=2, name=f"osb{i}")

        def em(oc, i=i, ob=ob):
            po = pb_sc.tile([128, CH], F32, tag="sc", name=f"po{i}_{oc}")
            nc.tensor.matmul(po, wo_sb[:, 128 * oc:128 * oc + 128],
                             y2T[:, CH * i:CH * i + CH], start=True, stop=True)
            if oc % 2 == 0:
                nc.vector.tensor_copy(out=ob[:, oc, :], in_=po)
            else:
                nc.scalar.copy(out=ob[:, oc, :], in_=po)
            if oc == 7:
                nc.sync.dma_start(out=outp_r[:, :, CH * i:CH * i + CH], in_=ob)
        return [lambda oc=oc: em(oc) for oc in range(8)]

    pending = []
    for i in range(NCH):
        yts = [pb_yt.tile([65, CH], F32, tag="yt", name=f"yt{i}_{h}")
               for h in (0, 1)]
        nj = 4 * i + 4

        # diagonal blocks first: their DVE mask-adds run while DVE is idle,
        # and the strip tail becomes a pure PE->ACT stream.
        order = list(range(4 * i, nj)) + list(range(0, 4 * i))

        def emit_pv(idx, j, ex, s0, yts=yts, i=i, nj=nj):
            first = (idx == 0)
            last_nd = (i > 0 and j == 4 * i - 1)
            for h in (0, 1):
                vst = v_ext[:, j, 65 * h:65 * h + 65]
                if i == 0:
                    # all-diag strip: stop on each leading 128 window
                    nc.tensor.matmul(yts[h][:, s0:s0 + 128], vst,
                                     ex[:, h, s0:s0 + 128],
                                     start=first, stop=True)
                    if s0 + 128 < CH:
                        nc.tensor.matmul(yts[h][:, s0 + 128:CH], vst,
                                         ex[:, h, s0 + 128:CH],
                                         start=first, stop=False)
                else:
                    nc.tensor.matmul(yts[h][:, s0:CH], vst, ex[:, h, s0:CH],
                                     start=first, stop=last_nd)

        prev = None
        for idx, j in enumerate(order):
            diag = (j - 4 * i) if j >= 4 * i else -1
            s0 = 128 * diag if diag >= 0 else 0
            sc = pb_sc.tile([128, 2, CH], F32, tag="sc", name=f"sc{i}_{j}")
            for h in (0, 1):
                kblk = k_fin[64 * h:64 * h + 64, 128 * j:128 * j + 128]
                if diag < 0:
                    nc.tensor.matmul(sc[:, h, :], kblk,
                                     q_fin[64 * h:64 * h + 64, CH * i:CH * i + CH],
                                     start=True, stop=True)
                else:
                    nc.tensor.matmul(sc[:, h, s0:s0 + 128], kblk,
                                     q_fin[64 * h:64 * h + 64,
                                           CH * i + s0:CH * i + s0 + 128],
                                     start=True, stop=False)
                    nc.tensor.matmul(sc[:, h, s0:s0 + 128], identb, maskb,
                                     start=False, stop=True)
                    if s0 + 128 < CH:
                        nc.tensor.matmul(sc[:, h, s0 + 128:CH], kblk,
                                         q_fin[64 * h:64 * h + 64,
                                               CH * i + s0 + 128:CH * i + CH],
                                         start=True, stop=True)
            ex = late.tile([128, 2, CH], BF16, tag="ex", bufs=3,
                           name=f"ex{i}_{j}")
            if SCHRAUD and idx % 3 == 1:
                nc.vector.tensor_scalar(
                    out=ex.bitcast(mybir.dt.int16)[:, :, s0:CH],
                    in0=sc[:, :, s0:CH],
                    scalar1=SA, scalar2=SB,
                    op0=mybir.AluOpType.mult, op1=mybir.AluOpType.add)
            else:
                nc.scalar.activation(out=ex[:, :, s0:CH], in_=sc[:, :, s0:CH],
                                     func=AF.Exp)
            if prev is not None:
                emit_pv(*prev)
            prev = (idx, j, ex, s0)
            if idx >= 2:
                for _ in range(2):
                    if pending:
                        pending.pop(0)()
        emit_pv(*prev)
        while pending:
            pending.pop(0)()

        invdbs = []
        for h in (0, 1):
            dens = late.tile([1, CH], F32, tag="dens", bufs=2,
                             name=f"dens{i}_{h}")
            nc.vector.tensor_copy(out=dens, in_=yts[h][64:65, :])
            invd = late.tile([1, CH], F32, tag="invd", bufs=2,
                             name=f"ivd{i}_{h}")
            nc.vector.reciprocal_approx_fast(out=invd, in_=dens)
            if dbg:
                nc.sync.dma_start(out=io["dbg_invd"].ap()[2 * i + h], in_=invd)
            invdb = late.tile([1, CH], BF16, tag="invdb", bufs=2,
                              name=f"ivdb{i}_{h}")
            nc.vector.tensor_copy(out=invdb, in_=invd)
            invdbs.append(invdb)

        def denfin(h, i=i, yts=yts, invdbs=invdbs):
            ib = pb_sc.tile([64, CH], F32, tag="sc", name=f"ib{i}_{h}")
            nc.tensor.matmul(ib, ones64, invdbs[h], start=True, stop=True)
            ibs = late.tile([64, CH], F32, tag="ibs", bufs=2, name=f"ibs{i}_{h}")
            nc.scalar.copy(out=ibs, in_=ib)
            nc.vector.tensor_mul(out=y2T[64 * h:64 * h + 64, CH * i:CH * i + CH],
                                 in0=yts[h][0:64, :], in1=ibs)

        pending = [lambda h=h: denfin(h) for h in (0, 1)] + make_oproj(i)

    for em in pending:
        em()

    if dbg:
        nc.sync.dma_start(out=io["dbg_q"].ap(), in_=q_fin)
        nc.sync.dma_start(out=io["dbg_k"].ap(), in_=k_fin)
        nc.sync.dma_start(out=io["dbg_vext"].ap(), in_=v_ext)
        nc.sync.dma_start(out=io["dbg_y2T"].ap(), in_=y2T)

    release(pb_yt)
    release(pb_sc)
    for p in reversed(pools):
        p.release()


_CACHE = {}


def _build(debug_taps=False):
    key = ("nc", debug_taps)
    if key in _CACHE:
        return _CACHE[key]
    nc = bacc.Bacc("TRN2", target_bir_lowering=False, debug=False,
                   enable_asserts=True, num_devices=NCORES)
    io = {}
    io["xT"] = nc.dram_tensor("xT", [D, S], BF16, kind="ExternalInput")
    io["cosT"] = nc.dram_tensor("cosT", [128, S], BF16, kind="ExternalInput")
    io["sinTs"] = nc.dram_tensor("sinTs", [128, S], BF16, kind="ExternalInput")
    io["identb"] = nc.dram_tensor("identb", [128, 128], BF16, kind="ExternalInput")
    io["maskb"] = nc.dram_tensor("maskb", [128, 128], BF16, kind="ExternalInput")
    io["c4q"] = nc.dram_tensor("c4q", [2, 2], F32, kind="ExternalInput")
    io["c4k"] = nc.dram_tensor("c4k", [2, 2], F32, kind="ExternalInput")
    io["ind8"] = nc.dram_tensor("ind8", [128, 4], BF16, kind="ExternalInput")
    io["indT2"] = nc.dram_tensor("indT2", [2, 128], BF16, kind="ExternalInput")
    io["ones64"] = nc.dram_tensor("ones64", [1, 64], BF16, kind="ExternalInput")
    io["wqP"] = nc.dram_tensor("wqP", [128, NKC, 128], BF16, kind="ExternalInput")
    io["wkP"] = nc.dram_tensor("wkP", [128, NKC, 128], BF16, kind="ExternalInput")
    io["wvP"] = nc.dram_tensor("wvP", [128, NKC, 128], BF16, kind="ExternalInput")
    io["woT"] = nc.dram_tensor("woT", [128, D], BF16, kind="ExternalInput")
    io["v1Ts"] = nc.dram_tensor("v1Ts", [128, S], BF16, kind="ExternalInput")
    io["outp"] = nc.dram_tensor("outp", [8, 128, S], BF16, kind="ExternalOutput")
    if debug_taps:
        io["dbg_q"] = nc.dram_tensor("dbg_q", [128, S], BF16, kind="ExternalOutput")
        io["dbg_k"] = nc.dram_tensor("dbg_k", [128, S], BF16, kind="ExternalOutput")
        io["dbg_vext"] = nc.dram_tensor("dbg_vext", [128, NB, 130], BF16, kind="ExternalOutput")
        io["dbg_y2T"] = nc.dram_tensor("dbg_y2T", [128, S], BF16, kind="ExternalOutput")
        io["dbg_invd"] = nc.dram_tensor("dbg_invd", [8, 1, CH], F32, kind="ExternalOutput")
        for j in (0, 4):
            io[f"dbg_ex{j}"] = nc.dram_tensor(f"dbg_ex{j}", [128, 2, CH], BF16, kind="ExternalOutput")
            io[f"dbg_sc{j}"] = nc.dram_tensor(f"dbg_sc{j}", [128, 2, CH], F32, kind="ExternalOutput")

    with tile.TileContext(nc) as tc:
        _emit(tc, io, dbg=debug_taps)
    nc.compile()
    _CACHE[key] = nc
    return nc


def _host_prep(x, v1, Wq, Wk, Wv, Wout, lambdas):
    """Build per-core input maps (bf16 numpy)."""
    import ml_dtypes
    BF = ml_dtypes.bfloat16

    x = np.asarray(x, np.float32).reshape(S, D)
    v1 = np.asarray(v1, np.float32).reshape(S, D)
    Wq = np.asarray(Wq, np.float32)
    Wk = np.asarray(Wk, np.float32)
    Wv = np.asarray(Wv, np.float32)
    Wout = np.asarray(Wout, np.float32)
    lam = np.float32(np.asarray(lambdas))

    xT = np.ascontiguousarray(x.T).astype(BF)

    inv_freq = (np.float32(1.0)
                / np.power(np.float32(10000.0),
                           np.arange(0, HD, 2, dtype=np.float32) / np.float32(HD)))
    t = np.arange(S, dtype=np.float32)
    freqs = np.outer(t, inv_freq).astype(np.float32)        # [S, 32]
    cos = np.cos(freqs).T                                    # [32, S]
    sin = np.sin(freqs).T
    cosT = np.ascontiguousarray(np.tile(cos, (4, 1))).astype(BF)
    sinTs = np.ascontiguousarray(
        np.concatenate([sin, -sin, sin, -sin], axis=0)).astype(BF)

    identb = np.eye(128, dtype=BF)
    kk, qq = np.meshgrid(np.arange(128), np.arange(128), indexing="ij")
    maskb = np.where(qq >= kk, 0.0, NEG).astype(BF)

    # sqrt(scale*sum + bias): rows 0-1 q -> 8*rms_q, rows 2-3 k -> rms_k
    c4q = np.array([[1.0, 64.0 * EPS],
                    [1.0, 64.0 * EPS]], dtype=np.float32)
    c4k = np.array([[1.0 / 64.0, EPS],
                    [1.0 / 64.0, EPS]], dtype=np.float32)
    ind8 = np.zeros((128, 4), dtype=BF)
    ind8[0:64, 0] = 1.0
    ind8[64:128, 1] = 1.0
    ind8[0:64, 2] = 1.0
    ind8[64:128, 3] = 1.0
    indT2 = np.zeros((2, 128), dtype=BF)
    indT2[0, 0:64] = 1.0
    indT2[1, 64:128] = 1.0
    ones64 = np.ones((1, 64), dtype=BF)

    shared = dict(xT=xT, cosT=cosT, sinTs=sinTs, identb=identb, maskb=maskb,
                  c4q=c4q, c4k=c4k, ind8=ind8, indT2=indT2, ones64=ones64)

    in_maps = []
    for c in range(NCORES):
        sl = slice(128 * c, 128 * c + 128)
        m = dict(shared)
        def prearr(wt):
            # [D, 128] -> [pi=128, po=8, m=128] so DMA runs are 2KB/partition
            return np.ascontiguousarray(
                wt.reshape(NKC, 128, 128).transpose(1, 0, 2)).astype(BF)

        m["wqP"] = prearr(Wq[sl, :].T)
        m["wkP"] = prearr(Wk[sl, :].T)
        m["wvP"] = prearr(((np.float32(1.0) - lam) * Wv[sl, :]).T)
        m["woT"] = np.ascontiguousarray(Wout[:, sl].T).astype(BF)
        m["v1Ts"] = np.ascontiguousarray((lam * v1[:, sl]).T).astype(BF)
        in_maps.append(m)
    return in_maps


def run(inputs, trace=False, debug_taps=False):
    nh = int(np.asarray(inputs["n_heads"]))
    assert nh == NH, f"kernel compiled for n_heads={NH}, got {nh}"
    nc = _build(debug_taps)
    in_maps = _host_prep(inputs["x"], inputs["v1"], inputs["Wq"], inputs["Wk"],
                         inputs["Wv"], inputs["Wout"], inputs["lambdas"])
    res = bass_utils.run_bass_kernel_spmd(
        nc, in_maps, core_ids=list(range(NCORES)), trace=trace)
    outT = np.zeros((D, S), dtype=np.float32)
    for c in range(NCORES):
        outT += np.asarray(res.results[c]["outp"], dtype=np.float32).reshape(D, S)
    y = np.ascontiguousarray(outT.T).reshape(1, S, D).astype(np.float32)
    v1 = np.asarray(inputs["v1"], np.float32).reshape(1, S, D)
    return (y, v1), res


def kernel(**inputs):
    outs, _ = run(inputs, trace=False)
    return outs
